# revision 48
# baseline (speedup 1.0000x reference)
"""GCN classifier (2x GCNConv + JK-cat + mean-pool + linear) on 8 trn2 NeuronCores.

v3 strategy. Dst-range sharding (each core owns the scatter-add for its
6250-node shard); the pipeline is built around the measured SWDGE gather
economics: per-queue in-order drain ~8ns per 256B row, 4 queues, so the whole
kernel is scheduled to keep all 4 gather queues continuously fed.

  - Layer 1 commutes projection with aggregation: x1 = relu((A_hat x) W1 + b1).
    Cores gather raw x rows (bf16 256B) from a host-provided node-major table;
    no AllGather, no pre-projection - gathers start at t~0.
  - Layer 2 aggregates x1 then projects with W2. x1 is AllGathered as padded
    256B bf16 rows in FOUR quarter-tables, each triggered as soon as the
    layer-1 blocks feeding it have flushed. Layer 1 processes dst blocks in
    order 6..12, 0..5 so the first quarters AllGather while layer 1 still
    drains; layer 2 consumes streams in table-readiness order (T2,T3,T0,T1)
    with the late tables' units deferred one block-pair.
  - Edges are packed in 128-edge chunks per (512-col dst block, src quarter),
    sorted by dst col; one dma_gather per (block, quarter) unit, round-robin
    over the 4 SWDGE queues; int16 gather indices address within a quarter.
  - Per chunk one PE matmul: psum[:, c0:c1] += msg^T @ S (S = edge norms at
    dst columns, streamed bf16). Self loops: ACT scaled copy (mult*dinv^2) +
    PE matmul against eye128 per 128-node window.
  - Block flush: psum -> bf16 -> W matmul -> relu(+b) -> xT; PE transposes
    build the node-major xc table ([x1|x2]) feeding the layer-2 diagonal, the
    AllGather staging rows, and the mean-pool matmuls. x1 pool partials run
    while layer-2 gathers drain; AllReduce + 128x7 linear finish.
"""
import numpy as np
import ml_dtypes

import concourse.bacc as bacc
import concourse.bass as bass
import concourse.mybir as mybir
import concourse.tile as tile
from concourse.bass_utils import run_bass_kernel_spmd

F32 = mybir.dt.float32
BF16 = mybir.dt.bfloat16
I16 = mybir.dt.int16
BF_NP = ml_dtypes.bfloat16

N, E, G = 50000, 800000, 64
D_IN, D_H, D_OUT = 128, 64, 7
NC = 8
SH = N // NC            # 6250 nodes per core
TILES = 49              # ceil(SH / 128)
SHP = TILES * 128       # 6272 padded shard rows
NP_ROWS = NC * SHP      # 50176 padded global rows (x table)
XQ = NP_ROWS // 4       # 12544 rows per x quarter-table
WIN = TILES             # 49 windows of 128 dst nodes
BLK_WINS = 4            # windows per psum block ([*, <=512])
NBLK = (WIN + BLK_WINS - 1) // BLK_WINS  # 13
NQ = 4                  # SWDGE queues
NSTR = 4                # src quarter-tables per layer

# layer-2 x1 half-tables, aligned to tile ranges (per-core rows)
TR = [(0, 24), (24, 49)]                            # xc tile ranges
L2_OFF = [t0 * 128 for t0, _ in TR]                 # 0, 3072
L2_SZ = [(t1 - t0) * 128 for t0, t1 in TR]          # 3072, 3200
NSTR2 = 2

# layer-1 dst blocks in natural order; table 0 (tiles 0-23 = blocks 0-5)
# stages + AllGathers mid-layer, table 1 (tiles 24-48) at the end
BORD1 = list(range(NBLK))
STAGE_AT = {5: 0, 12: 1}     # flush of block b -> stage x1 table k
TRIG_LAG = 12                # units between a stage and its AG trigger

_cache = {}


def _l1_units():
    return [(b, s) for b in BORD1 for s in range(NSTR)]


def _l1_triggers(units_bt):
    """(unit_pos, table) pairs: trigger emitted after unit_pos's gather.

    pos -1 = emit after the unit loop (must follow staging-DMA emission)."""
    last = {}
    for i, (b, s) in enumerate(units_bt):
        last[b] = i
    out = []
    for b, k in STAGE_AT.items():
        pos = last[b] + TRIG_LAG
        out.append((pos if pos < len(units_bt) else -1, k))
    return sorted(out)


def _l2_units():
    # phase A: table-0 edges, partials banked to SBUF; phase B: table-1
    # edges + diagonal, dual-projection merge on flush. Each (block, table)
    # is split into two gather units for finer queue interleave.
    out = []
    for t in range(NSTR2):
        for b in range(NBLK):
            out.append((b, t))
            out.append((b, t))
    return out


def _schedule_layer(src, dst, norm, stream, idx16, units_bt, nstr):
    """Pack one layer's edges into per-(core, block, stream) 128-edge chunks.

    stream/idx16: per-edge table index and int16 row within that table.
    units_bt: explicit (block, stream) unit order.
    Returns (meta, idx_w [NC,128,nslot*8] i16, s_tab [NC,128,stot] bf16).
    """
    core = dst // SH
    l = dst - core * SH
    blk = l // (BLK_WINS * 128)

    per = [[[None] * nstr for _ in range(NBLK)] for _ in range(NC)]
    nch = np.zeros((NBLK, nstr), np.int64)
    for c in range(NC):
        m = core == c
        lc, sc, ic, nc_, bc = l[m], stream[m], idx16[m], norm[m], blk[m]
        perm = np.lexsort((ic, lc, sc, bc))
        lc, sc, ic, nc_, bc = (lc[perm], sc[perm], ic[perm], nc_[perm],
                               bc[perm])
        for b in range(NBLK):
            bm = bc == b
            for t in range(nstr):
                tm = bm & (sc == t)
                ne = int(tm.sum())
                per[c][b][t] = (lc[tm], ic[tm], nc_[tm])
                nch[b, t] = max(nch[b, t], (ne + 127) // 128)

    from collections import Counter
    occ = Counter(units_bt)
    seen = Counter()
    slots = []           # (b, t, k)
    units = []           # (b, t, slot_lo, slot_hi)
    for b, t in units_bt:
        n_tot = int(nch[b, t])
        i = seen[(b, t)]
        seen[(b, t)] += 1
        lo = (n_tot * i) // occ[(b, t)]
        hi = (n_tot * (i + 1)) // occ[(b, t)]
        s0 = len(slots)
        for k in range(lo, hi):
            slots.append((b, t, k))
        if len(slots) > s0:
            units.append((b, t, s0, len(slots)))
    nslot = len(slots)

    slot_pos = {s: i for i, s in enumerate(slots)}
    c0s = np.full(nslot, 1 << 30, np.int64)
    c1s = np.full(nslot, -1, np.int64)
    for c in range(NC):
        for b in range(NBLK):
            for t in range(nstr):
                lc = per[c][b][t][0]
                for k in range((len(lc) + 127) // 128):
                    si = slot_pos[(b, t, k)]
                    seg = lc[k * 128:(k + 1) * 128] - b * BLK_WINS * 128
                    c0s[si] = min(c0s[si], seg.min())
                    c1s[si] = max(c1s[si], seg.max() + 1)
    c0s = np.where(c1s < 0, 0, c0s)
    c1s = np.maximum(c1s, c0s + 1)
    ms = c1s - c0s
    s_off = np.zeros(nslot + 1, np.int64)
    s_off[1:] = np.cumsum(ms)
    stot = int(s_off[-1])

    idx_flat = np.zeros((NC, nslot * 128), np.int16)  # pad idx 0 (S row = 0)
    s_tab = np.zeros((NC, 128, stot), np.float32)
    for c in range(NC):
        for b in range(NBLK):
            for t in range(nstr):
                lc, ic, nc_ = per[c][b][t]
                for k in range((len(lc) + 127) // 128):
                    si = slot_pos[(b, t, k)]
                    sl = slice(k * 128, min((k + 1) * 128, len(lc)))
                    n_in = sl.stop - sl.start
                    idx_flat[c, si * 128: si * 128 + n_in] = ic[sl]
                    rel = lc[sl] - b * BLK_WINS * 128 - c0s[si]
                    s_tab[c, np.arange(n_in), s_off[si] + rel] = nc_[sl]

    idx_w = np.zeros((NC, 128, nslot * 8), np.int16)
    for c in range(NC):
        w = idx_flat[c].reshape(-1, 16).T
        idx_w[c] = np.tile(w, (8, 1))

    meta = dict(slots=slots, units=units, ms=ms, s_off=s_off, stot=stot,
                col0=c0s.copy(), nslot=nslot)
    return meta, idx_w, s_tab.astype(BF_NP)


def _build(meta1, meta2):
    nc = bacc.Bacc("TRN2", target_bir_lowering=False, debug=False,
                   num_devices=NC, num_swdge_queues=NQ)

    xbf_d = nc.dram_tensor("xbf", [NP_ROWS, D_IN], BF16, kind="ExternalInput")
    xsh_d = nc.dram_tensor("xsh", [SHP, D_IN], BF16, kind="ExternalInput")
    idx1_d = nc.dram_tensor("idx1", [128, meta1["nslot"] * 8], I16,
                            kind="ExternalInput")
    idx2_d = nc.dram_tensor("idx2", [128, meta2["nslot"] * 8], I16,
                            kind="ExternalInput")
    s1_d = nc.dram_tensor("s1", [128, meta1["stot"]], BF16,
                          kind="ExternalInput")
    s2_d = nc.dram_tensor("s2", [128, meta2["stot"]], BF16,
                          kind="ExternalInput")
    spool_d = nc.dram_tensor("spool", [128, TILES, G], BF16,
                             kind="ExternalInput")
    dinv2_d = nc.dram_tensor("dinv2", [128, TILES], F32, kind="ExternalInput")
    w1_d = nc.dram_tensor("W1", [D_IN, D_H], BF16, kind="ExternalInput")
    w2_d = nc.dram_tensor("W2", [D_H, D_H], BF16, kind="ExternalInput")
    wl_d = nc.dram_tensor("Wlin", [2 * D_H, D_OUT], F32, kind="ExternalInput")
    b1_d = nc.dram_tensor("b1", [D_H, 1], F32, kind="ExternalInput")
    b2_d = nc.dram_tensor("b2", [D_H, 1], F32, kind="ExternalInput")
    bl_d = nc.dram_tensor("blin_t", [G, D_OUT], F32, kind="ExternalInput")
    eye64_d = nc.dram_tensor("eye64", [D_H, D_H], BF16, kind="ExternalInput")
    eye128_d = nc.dram_tensor("eye128", [128, 128], BF16,
                              kind="ExternalInput")
    out_d = nc.dram_tensor("out", [G, D_OUT], F32, kind="ExternalOutput")

    x1_loc = [nc.dram_tensor(f"x1_loc{k}", [L2_SZ[k], D_IN], BF16)
              for k in range(NSTR2)]
    x1_full = [nc.dram_tensor(f"x1_full{k}", [NC * L2_SZ[k], D_IN], BF16,
                              addr_space="Shared") for k in range(NSTR2)]
    pool_loc = nc.dram_tensor("pool_loc", [128, G], F32)
    pool_full = nc.dram_tensor("pool_full", [128, G], F32, addr_space="Shared")

    max_u = max(max(u[3] - u[2] for u in m["units"]) for m in (meta1, meta2))
    max_s = max(max(int(m["s_off"][u[3]] - m["s_off"][u[2]])
                    for u in m["units"]) for m in (meta1, meta2))
    blk_w = [min((b + 1) * BLK_WINS, WIN) * 128 - b * BLK_WINS * 128
             for b in range(NBLK)]
    trig1 = _l1_triggers([(u[0], u[1]) for u in meta1["units"]])

    with tile.TileContext(nc) as tc:
        with (
            tc.tile_pool(name="persist", bufs=1) as pp,
            tc.tile_pool(name="msg", bufs=9) as mpool,
            tc.tile_pool(name="stabp", bufs=8) as spool_p,
            tc.tile_pool(name="selfp", bufs=3) as selfp,
            tc.tile_pool(name="aggsb", bufs=2) as aggsb,
            tc.tile_pool(name="psAgg", bufs=4, space="PSUM") as psAgg,
            tc.tile_pool(name="psProj", bufs=1, space="PSUM") as psProj,
            tc.tile_pool(name="psTrans", bufs=1, space="PSUM") as psTrans,
            tc.tile_pool(name="psPool", bufs=1, space="PSUM") as psPool,
        ):
            w1_t = pp.tile([D_IN, D_H], BF16)
            w2_t = pp.tile([D_H, D_H], BF16)
            wl_t = pp.tile([2 * D_H, D_OUT], F32)
            b_t = [pp.tile([D_H, 1], F32, name=f"b{i}", tag=f"b{i}")
                   for i in range(2)]
            bl_t = pp.tile([G, D_OUT], F32)
            eye64_t = pp.tile([D_H, D_H], BF16)
            eye128_t = pp.tile([128, 128], BF16)
            dinv2_t = pp.tile([128, TILES], F32)
            zz_t = pp.tile([128, 512], BF16)
            idx_t = [pp.tile([128, m["nslot"] * 8], I16, name=f"idx{i}",
                             tag=f"idx{i}")
                     for i, m in ((0, meta1), (1, meta2))]
            spool_t = pp.tile([128, TILES, G], BF16)
            xnode = pp.tile([128, TILES, D_IN], BF16)
            xc_t = pp.tile([128, TILES, 2 * D_H], BF16)
            xT_t = [pp.tile([D_H, SHP], BF16, name=f"xT{i}", tag=f"xT{i}")
                    for i in range(2)]
            # layer-2 phase-A per-block partial aggregates (bf16)
            aggPart = pp.tile([D_H, NBLK, 512], BF16)

            # header loads; first-wave idx slices first so gathers start fast
            n1 = meta1["nslot"] * 8
            u0 = meta1["units"][3][3] * 8   # first 4 units
            nc.sync.dma_start(idx_t[0][:, 0:u0], idx1_d[:, 0:u0])
            iq = (n1 - u0 + 2) // 3
            for q in range(3):
                i0 = u0 + q * iq
                i1 = min(u0 + (q + 1) * iq, n1)
                if i1 > i0:
                    nc.sync.dma_start(idx_t[0][:, i0:i1], idx1_d[:, i0:i1])
            nc.scalar.dma_start(idx_t[1][:], idx2_d[:])
            nc.sync.dma_start(w1_t[:], w1_d[:])
            nc.sync.dma_start(w2_t[:], w2_d[:])
            nc.sync.dma_start(wl_t[:], wl_d[:])
            nc.sync.dma_start(b_t[0][:], b1_d[:])
            nc.sync.dma_start(b_t[1][:], b2_d[:])
            nc.sync.dma_start(bl_t[:], bl_d[:])
            nc.sync.dma_start(eye64_t[:], eye64_d[:])
            nc.sync.dma_start(eye128_t[:], eye128_d[:])
            nc.sync.dma_start(dinv2_t[:], dinv2_d[:])
            nc.scalar.dma_start(spool_t[:], spool_d[:])
            nc.vector.memset(zz_t[:], 0.0)
            nc.vector.memset(xc_t[:], 0.0)

            # own-shard node-major x rows for the self-loop diagonal
            for q in range(4):
                t0, t1 = q * 13, min((q + 1) * 13, TILES)
                nc.scalar.dma_start(
                    xnode[:, t0:t1, :],
                    xsh_d[t0 * 128:t1 * 128, :].rearrange(
                        "(t p) f -> p t f", p=128))

            def layer(L):
                meta = meta1 if L == 0 else meta2
                units, ms, s_off, col0 = (meta["units"], meta["ms"],
                                          meta["s_off"], meta["col0"])
                s_d = s1_d if L == 0 else s2_d
                feat = D_IN if L == 0 else D_H
                if L == 0:
                    tables = [xbf_d[k * XQ:(k + 1) * XQ, :]
                              for k in range(NSTR)]
                else:
                    tables = [x1_full[k][:] for k in range(NSTR2)]

                ps_blk = [None] * NBLK
                done_units = [0] * NBLK
                units_per_blk = [0] * NBLK
                done_bt = {}
                units_per_bt = {}
                for (b, t, a0, a1) in units:
                    units_per_blk[b] += 1
                    units_per_bt[(b, t)] = units_per_bt.get((b, t), 0) + 1

                def open_block(b, diag):
                    ps = psAgg.tile([128, 512], F32, name="psb", tag="psb")
                    nc.tensor.matmul(ps[0:feat, 0:blk_w[b]], zz_t[:, 0:feat],
                                     zz_t[:, 0:blk_w[b]], start=True,
                                     stop=True)
                    if not diag:
                        return ps
                    for wi in range(b * BLK_WINS,
                                    min((b + 1) * BLK_WINS, WIN)):
                        sm = selfp.tile([128, D_IN], BF16, name="selfm",
                                        tag="selfm")
                        src_tile = (xnode[:, wi, :] if L == 0
                                    else xc_t[:, wi, 0:D_H])
                        nc.scalar.activation(
                            sm[:, 0:feat], src_tile,
                            mybir.ActivationFunctionType.Copy,
                            scale=dinv2_t[:, wi:wi + 1])
                        c0w = (wi - b * BLK_WINS) * 128
                        nc.tensor.matmul(
                            ps[0:feat, c0w:c0w + 128], sm[:, 0:feat],
                            eye128_t[:], start=False, stop=True,
                            skip_group_check=True)
                    return ps

                def stage1_flush(b):
                    # bank phase-A partial aggregate; frees the psum tile
                    ps = ps_blk[b]
                    nc.scalar.copy(aggPart[:, b, 0:blk_w[b]],
                                   ps[0:D_H, 0:blk_w[b]])
                    ps_blk[b] = None

                def flush_block(b):
                    ps = ps_blk[b]
                    bw = blk_w[b]
                    agg = aggsb.tile([128, 512], BF16, name="aggT",
                                     tag="aggT")
                    nc.scalar.copy(agg[0:feat, 0:bw], ps[0:feat, 0:bw])
                    ps2 = psProj.tile([D_H, 512], F32, name="proj",
                                      tag="proj")
                    wt = w1_t if L == 0 else w2_t
                    if L == 0:
                        nc.tensor.matmul(ps2[:, 0:bw], wt[0:feat, :],
                                         agg[0:feat, 0:bw])
                    else:
                        nc.tensor.matmul(ps2[:, 0:bw], wt[0:feat, :],
                                         agg[0:feat, 0:bw], start=True,
                                         stop=False)
                        nc.tensor.matmul(ps2[:, 0:bw], wt[0:feat, :],
                                         aggPart[:, b, 0:bw], start=False,
                                         stop=True)
                    w0 = b * BLK_WINS * 128
                    xT = xT_t[L]
                    nc.scalar.activation(
                        xT[:, w0:w0 + bw], ps2[:, 0:bw],
                        mybir.ActivationFunctionType.Relu, bias=b_t[L])
                    for ti in range(b * BLK_WINS,
                                    min((b + 1) * BLK_WINS, WIN)):
                        pst = psTrans.tile([128, D_H], BF16, name="pst",
                                           tag="pst")
                        nc.tensor.transpose(
                            pst[:], xT[:, ti * 128:(ti + 1) * 128],
                            eye64_t[:])
                        nc.scalar.copy(
                            xc_t[:, ti, L * D_H:(L + 1) * D_H], pst[:])
                        if L == 1:
                            nc.tensor.matmul(
                                pool_ps[D_H:2 * D_H, :],
                                xc_t[:, ti, D_H:2 * D_H],
                                spool_t[:, ti, :], start=(ti == 0),
                                stop=(ti == WIN - 1),
                                skip_group_check=True)
                    if L == 0 and b in STAGE_AT:
                        k = STAGE_AT[b]
                        t0, t1 = TR[k]
                        nc.sync.dma_start(
                            x1_loc[k][:].rearrange("(t p) f -> p t f",
                                                   p=128),
                            xc_t[:, t0:t1, :])

                trig = (dict((p, k) for p, k in trig1 if p >= 0)
                        if L == 0 else {})
                tail_trig = ([k for p, k in trig1 if p < 0]
                             if L == 0 else [])
                qn = 0
                for ui, (b, t, a0, a1) in enumerate(units):
                    nh = a1 - a0
                    mt = mpool.tile([128, max_u, D_IN], BF16, name="msg",
                                    tag="msg")
                    nc.gpsimd.dma_gather(
                        mt[:, 0:nh, :], tables[t],
                        idx_t[L][:, a0 * 8:a1 * 8],
                        nh * 128, nh * 128, D_IN,
                        single_packet=False, queue_num=qn % NQ)
                    qn += 1
                    if ui in trig:
                        k = trig[ui]
                        nc.gpsimd.collective_compute(
                            "AllGather", mybir.AluOpType.bypass,
                            replica_groups=[list(range(NC))],
                            ins=[x1_loc[k][:]], outs=[x1_full[k][:]])
                    st_t = spool_p.tile([128, max_s], BF16, name="stab",
                                        tag="stab")
                    u_soff = int(s_off[a0])
                    u_slen = int(s_off[a1] - u_soff)
                    nc.sync.dma_start(st_t[:, 0:u_slen],
                                      s_d[:, u_soff:u_soff + u_slen])
                    if ps_blk[b] is None:
                        # L2 phase A (t==0) defers the diagonal to phase B
                        ps_blk[b] = open_block(b, L == 0 or t == 1)
                    for si in range(a0, a1):
                        m = int(ms[si])
                        so = int(s_off[si] - u_soff)
                        c0 = int(col0[si])
                        nc.tensor.matmul(
                            ps_blk[b][0:feat, c0:c0 + m],
                            mt[:, si - a0, 0:feat],
                            st_t[:, so:so + m],
                            start=False, stop=True, skip_group_check=True)
                    done_units[b] += 1
                    done_bt[(b, t)] = done_bt.get((b, t), 0) + 1
                    if L == 1:
                        if done_bt[(b, t)] == units_per_bt[(b, t)]:
                            if t == 0:
                                stage1_flush(b)
                            else:
                                flush_block(b)
                    elif done_units[b] == units_per_blk[b]:
                        flush_block(b)
                for k in tail_trig:
                    nc.gpsimd.collective_compute(
                        "AllGather", mybir.AluOpType.bypass,
                        replica_groups=[list(range(NC))],
                        ins=[x1_loc[k][:]], outs=[x1_full[k][:]])

            pool_ps = psPool.tile([128, G], F32, name="poolps", tag="poolps",
                                  bufs=1)
            layer(0)
            # x1 pool partials run on PE while layer-2 gathers drain
            for ti in range(TILES):
                nc.tensor.matmul(pool_ps[0:D_H, :], xc_t[:, ti, 0:D_H],
                                 spool_t[:, ti, :], start=(ti == 0),
                                 stop=(ti == TILES - 1),
                                 skip_group_check=True)
            layer(1)

            pool_sb = pp.tile([128, G], F32)
            nc.scalar.copy(pool_sb[:], pool_ps[:])
            nc.sync.dma_start(pool_loc[:], pool_sb[:])
            nc.gpsimd.collective_compute(
                "AllReduce", mybir.AluOpType.add,
                replica_groups=[list(range(NC))],
                ins=[pool_loc[:]], outs=[pool_full[:]])
            pooled_t = pp.tile([128, G], F32)
            nc.sync.dma_start(pooled_t[:], pool_full[:])
            fin_ps = psPool.tile([G, D_OUT], F32, name="fin", tag="fin",
                                 bufs=1)
            nc.tensor.matmul(fin_ps[:], pooled_t[:], wl_t[:])
            out_t = pp.tile([G, D_OUT], F32)
            nc.vector.tensor_add(out_t[:], fin_ps[:], bl_t[:])
            nc.sync.dma_start(out_d[:], out_t[:])

    nc.compile()
    return nc


def _prep_inputs(x, edge_index, batch, W1, b1, W2, b2, Wlin, blin):
    src = np.asarray(edge_index[0]).astype(np.int64)
    dst = np.asarray(edge_index[1]).astype(np.int64)
    src_all = np.concatenate([src, np.arange(N, dtype=np.int64)])
    dst_all = np.concatenate([dst, np.arange(N, dtype=np.int64)])
    deg = np.bincount(dst_all, minlength=N).astype(np.float64)
    dinv = 1.0 / np.sqrt(np.maximum(deg, 1e-12))
    batch_np = np.asarray(batch).astype(np.int64)

    keep = src != dst
    srck, dstk = src[keep], dst[keep]
    normk = (dinv[srck] * dinv[dstk]).astype(np.float32)

    # layer 1: quarter-tables split by global row (padded table NP_ROWS)
    st1 = np.minimum(srck // XQ, NSTR - 1)
    i16_1 = (srck - st1 * XQ).astype(np.int16)
    meta1, idx1_w, s1_tab = _schedule_layer(srck, dstk, normk, st1, i16_1,
                                            _l1_units(), NSTR)

    # layer 2: half-tables split by tile range within each shard
    srcc = srck // SH
    srcr = srck - srcc * SH
    st2 = (srcr >= L2_OFF[1]).astype(np.int64)
    i16_2 = (srcc * np.array(L2_SZ)[st2] + srcr -
             np.array(L2_OFF)[st2]).astype(np.int16)
    meta2, idx2_w, s2_tab = _schedule_layer(srck, dstk, normk, st2, i16_2,
                                            _l2_units(), NSTR2)

    x = np.asarray(x, np.float32)
    xbf = np.zeros((NP_ROWS, D_IN), BF_NP)
    xbf[0:N] = x.astype(BF_NP)

    # diagonal weight: (# self edges incl. added loop) * dinv^2
    mult = 1.0 + np.bincount(dst[src == dst], minlength=N).astype(np.float64)
    dval = (mult * dinv * dinv).astype(np.float32)
    dinv2 = np.zeros((NC, 128, TILES), np.float32)
    r = np.arange(SH)
    for c in range(NC):
        dinv2[c, r % 128, r // 128] = dval[c * SH + r]

    cnt = np.maximum(np.bincount(batch_np, minlength=G), 1).astype(np.float32)
    s_pool = np.zeros((NC, 128, TILES, G), np.float32)
    for c in range(NC):
        g = batch_np[c * SH:(c + 1) * SH]
        s_pool[c, r % 128, r // 128, g] = 1.0 / cnt[g]

    com = dict(
        xbf=xbf,
        W1=np.asarray(W1, np.float32).astype(BF_NP),
        W2=np.asarray(W2, np.float32).astype(BF_NP),
        Wlin=np.asarray(Wlin, np.float32),
        b1=np.asarray(b1, np.float32).reshape(D_H, 1),
        b2=np.asarray(b2, np.float32).reshape(D_H, 1),
        blin_t=np.tile(np.asarray(blin, np.float32), (G, 1)),
        eye64=np.eye(D_H, dtype=BF_NP),
        eye128=np.eye(128, dtype=BF_NP),
    )
    in_maps = [
        dict(com, xsh=np.ascontiguousarray(xbf[c * SH:c * SH + SHP]),
             idx1=idx1_w[c], idx2=idx2_w[c], s1=s1_tab[c],
             s2=s2_tab[c], spool=s_pool[c].astype(BF_NP), dinv2=dinv2[c])
        for c in range(NC)
    ]
    return meta1, meta2, in_maps


def kernel(x, edge_index, batch, W1, b1, W2, b2, Wlin, blin, _trace=False):
    meta1, meta2, in_maps = _prep_inputs(x, edge_index, batch, W1, b1, W2,
                                         b2, Wlin, blin)
    key = (meta1["nslot"], meta1["stot"], meta2["nslot"], meta2["stot"],
           tuple(meta1["ms"].tolist()), tuple(meta2["ms"].tolist()))
    if key not in _cache:
        _cache.clear()
        _cache[key] = _build(meta1, meta2)
    nc = _cache[key]
    res = run_bass_kernel_spmd(nc, in_maps, list(range(NC)), trace=_trace)
    out = res.results[0]["out"].astype(np.float32)
    if _trace:
        return out, res.exec_time_ns
    return out
